# revision 17
# baseline (speedup 1.0000x reference)
"""Trainium2 Bass kernel for nn_ARCGridGNNEncoder.

Full-input contract: kernel(**inputs) takes the complete unsharded inputs
(as produced by the problem's setup_inputs) and returns the full output
[B, 900, 64] float32.  Internally shards the batch dim across 8 NeuronCores
(pure data parallel), runs a Bass/Tile kernel per core, gathers the result.

Math notes (host-side folds):
  - GCN aggregation  agg[dst] = sum_e dinv[src]*dinv[dst]*h[src]  is a fixed
    sparse matrix Ahat (symmetric).  We pad nodes 900->1024 and store Ahat as
    banded 128x128 blocks; aggregation becomes PE matmuls with the sample's
    node-chunk as the stationary operand (lhsT = x_chunk -> output lands in
    hid-major layout), then the Wg matmul flips back to node-major.
  - LayerNorm mean subtraction is folded into the weights: Wgc = Wg @ (I-J/128)
    so matmul output is already (approximately exactly) mean-centered per node.
    Variance comes from grouped bn_stats; since rstd > 0 and (gamma==1,beta==0)
    relu(LN(agg)) == rstd * relu(agg_c), applied as a per-partition
    tensor_scalar multiply plus the residual add.
  - bin_ is folded into the encoder matmul via an appended ones-row.
    Non-trivial bg/gamma/beta/bout are supported via extra ops that are only
    emitted when the actual input values need them.
"""

import os
import sys

import numpy as np

for _p in ("/opt/trn_rl_repo",):
    if _p not in sys.path:
        sys.path.insert(0, _p)

import ml_dtypes

B, H, W = 512, 30, 30
N = H * W  # 900
NUM_COLORS, IN_DIM, HID, OUT_DIM, NUM_LAYERS = 10, 12, 128, 64, 4
LN_EPS = 1e-5
N_CORES = 8
S = B // N_CORES  # 64 samples per core
NCH = 8  # node chunks
NPAD = NCH * 128  # 1024 padded nodes

BF16 = ml_dtypes.bfloat16


# --------------------------------------------------------------------------
# Host-side constant preparation
# --------------------------------------------------------------------------
def prep_consts(edge_index, Win, bin_, Wg, bg, gamma, beta, Wout, bout):
    edge_index = np.asarray(edge_index)
    Win = np.asarray(Win, np.float64)
    bin_ = np.asarray(bin_, np.float64)
    Wg = np.asarray(Wg, np.float64)
    bg = np.asarray(bg, np.float64)
    gamma = np.asarray(gamma, np.float64)
    beta = np.asarray(beta, np.float64)
    Wout = np.asarray(Wout, np.float64)
    bout = np.asarray(bout, np.float64)

    src, dst = edge_index[0].astype(np.int64), edge_index[1].astype(np.int64)
    deg = np.zeros(N, np.float64)
    np.add.at(deg, dst, 1.0)
    dinv = 1.0 / np.sqrt(np.maximum(deg, 1.0))

    Ahat = np.zeros((NPAD, NPAD), np.float64)
    np.add.at(Ahat, (dst, src), dinv[src] * dinv[dst])
    # blocks: Ablk[(I,J)] = Ahat[dst chunk J, src chunk I] laid out as
    # [128 src(partition k), 128 dst(col)] so matmul lhsT=x_I, rhs=blk -> vT.
    blocks = {}
    for I in range(NCH):
        for J in range(NCH):
            blk = Ahat[J * 128 : (J + 1) * 128, I * 128 : (I + 1) * 128].T
            if np.any(blk != 0.0):
                blocks[(I, J)] = blk
    blk_list = sorted(blocks.keys())
    ablk = np.zeros((128, len(blk_list) * 128), np.float64)
    blk_idx = {}
    for i, key in enumerate(blk_list):
        ablk[:, i * 128 : (i + 1) * 128] = blocks[key]
        blk_idx[key] = i

    # centering fold: Wgc = Wg @ (I - 11^T/128)
    C = np.eye(HID) - np.ones((HID, HID)) / HID
    wgc = np.concatenate([Wg[l] @ C for l in range(NUM_LAYERS)], axis=1)  # [128, 4*128]

    # encoder: featT rows 0..9 onehot, 10..11 pos, 12 ones (bin_ fold)
    rows = np.repeat(np.arange(H, dtype=np.float64) / max(H - 1, 1), W)
    cols = np.tile(np.arange(W, dtype=np.float64) / max(W - 1, 1), H)
    pos3 = np.zeros((3, NPAD), np.float64)
    pos3[0, :N] = rows
    pos3[1, :N] = cols
    pos3[2, :N] = 1.0
    win_aug = np.zeros((13, HID), np.float64)
    win_aug[:12] = Win
    win_aug[12] = bin_

    ident = np.eye(128, dtype=np.float64)

    bgc = bg - bg.mean(axis=1, keepdims=True)  # [L, 128] centered
    need_bg = bool(np.any(np.abs(bgc) > 0))
    need_gb = bool(np.any(np.abs(gamma - 1.0) > 0) or np.any(np.abs(beta) > 0))
    need_bout = bool(np.any(np.abs(bout) > 0))

    win4 = np.zeros((128, HID), np.float64)
    win34 = np.zeros((128, HID), np.float64)
    pos34 = np.zeros((128, NPAD), np.float64)
    for q in range(4):
        win4[32 * q : 32 * q + NUM_COLORS] = win_aug[:NUM_COLORS]
        win34[32 * q : 32 * q + 3] = win_aug[NUM_COLORS:13]
        pos34[32 * q : 32 * q + 3] = pos3
    consts = {
        "ablk": ablk.astype(BF16),
        "wgc": wgc.astype(BF16),
        "win": win_aug.astype(BF16),
        "pos3": pos3.astype(BF16),
        "win4": win4.astype(BF16),
        "win34": win34.astype(BF16),
        "pos34": pos34.astype(BF16),
        "wout": Wout.astype(BF16),
        "ident": ident.astype(BF16),
    }
    meta = {
        "blk_idx": blk_idx,
        "n_blk": len(blk_list),
        "need_bg": need_bg,
        "need_gb": need_gb,
        "need_bout": need_bout,
    }
    if need_bg:
        consts["bgc"] = np.ascontiguousarray(bgc.astype(BF16))  # [L,128]
        consts["ones1"] = np.ones((1, 128), BF16)
    if need_gb:
        consts["gam_b"] = np.tile(gamma.astype(np.float32), (1, NCH))  # [L, 1024]
        consts["bet_b"] = np.tile(beta.astype(np.float32), (1, NCH))
    if need_bout:
        consts["bout"] = bout.astype(np.float32).reshape(OUT_DIM, 1)
    return consts, meta


def golden_host(grids_flat, consts, meta):
    """Numpy model of exactly what the device kernel computes (fp32-ish).
    grids_flat: [nS, 900] int32. Returns [nS, 900, 64] float32."""
    ablk = consts["ablk"].astype(np.float32)
    blk_idx = meta["blk_idx"]
    Ahat = np.zeros((NPAD, NPAD), np.float32)
    for (I, J), i in blk_idx.items():
        Ahat[J * 128 : (J + 1) * 128, I * 128 : (I + 1) * 128] = ablk[
            :, i * 128 : (i + 1) * 128
        ].T
    wgc = consts["wgc"].astype(np.float32)
    win = consts["win"].astype(np.float32)
    pos3 = consts["pos3"].astype(np.float32)
    wout = consts["wout"].astype(np.float32)
    outs = []
    for g in grids_flat:
        featT = np.zeros((13, NPAD), np.float32)
        for c in range(NUM_COLORS):
            featT[c, :N] = (g == c).astype(np.float32)
        featT[10:13] = pos3
        x = np.maximum(featT.T @ win, 0.0)  # [1024, 128]
        x = x.astype(BF16).astype(np.float32)
        for l in range(NUM_LAYERS):
            v = (Ahat.astype(np.float32) @ x).astype(BF16).astype(np.float32)
            aggc = v @ wgc[:, l * 128 : (l + 1) * 128]
            if meta["need_bg"]:
                aggc = aggc + consts["bgc"][l].astype(np.float32)
            m1 = aggc[:, 0::2].mean(axis=1, keepdims=True)
            m2 = aggc[:, 1::2].mean(axis=1, keepdims=True)
            s1 = aggc[:, 0::2].var(axis=1, keepdims=True) * 64
            s2 = aggc[:, 1::2].var(axis=1, keepdims=True) * 64
            var = (s1 + s2 + 64 * (m1 * m1 + m2 * m2)) / 128 - ((m1 + m2) / 2) ** 2
            rstd = 1.0 / np.sqrt(var + LN_EPS)
            if meta["need_gb"]:
                t = aggc * rstd * consts["gam_b"][l][:128] + consts["bet_b"][l][:128]
                r = np.maximum(t, 0.0)
                x = (x + r.astype(BF16).astype(np.float32)).astype(BF16).astype(np.float32)
            else:
                r = np.maximum(aggc, 0.0).astype(BF16).astype(np.float32)
                x = (x + (r * rstd).astype(BF16).astype(np.float32)).astype(BF16).astype(
                    np.float32
                )
        x = x.astype(BF16).astype(np.float32)
        o = x @ wout  # [1024, 64]
        if meta["need_bout"]:
            o = o + consts["bout"].reshape(1, OUT_DIM)
        outs.append(o[:N])
    return np.stack(outs)


# --------------------------------------------------------------------------
# Bass kernel build
# --------------------------------------------------------------------------
def build_nc(consts, meta):
    import concourse.bass as bass
    import concourse.mybir as mybir
    import concourse.tile as tile

    f32 = mybir.dt.float32
    bf16 = mybir.dt.bfloat16
    i32 = mybir.dt.int32
    AF = mybir.ActivationFunctionType
    ALU = mybir.AluOpType

    nc = bass.Bass()

    grids_d = nc.declare_dram_parameter("grids", [128, (S // 4) * NPAD], bf16, isOutput=False)
    ablk_d = nc.declare_dram_parameter(
        "ablk", list(consts["ablk"].shape), bf16, isOutput=False
    )
    wgc_d = nc.declare_dram_parameter("wgc", [128, NUM_LAYERS * 128], bf16, isOutput=False)
    win4_d = nc.declare_dram_parameter("win4", [128, HID], bf16, isOutput=False)
    win34_d = nc.declare_dram_parameter("win34", [128, HID], bf16, isOutput=False)
    pos34_d = nc.declare_dram_parameter("pos34", [128, NPAD], bf16, isOutput=False)
    wout_d = nc.declare_dram_parameter("wout", [HID, OUT_DIM], bf16, isOutput=False)
    ident_d = nc.declare_dram_parameter("ident", [128, 128], bf16, isOutput=False)
    out_d = nc.declare_dram_parameter("out", [S, OUT_DIM, N], f32, isOutput=True)

    extra_d = {}
    if meta["need_bg"]:
        extra_d["bgc"] = nc.declare_dram_parameter(
            "bgc", [NUM_LAYERS, 128], bf16, isOutput=False
        )
        extra_d["ones1"] = nc.declare_dram_parameter("ones1", [1, 128], bf16, isOutput=False)
    if meta["need_gb"]:
        extra_d["gam_b"] = nc.declare_dram_parameter(
            "gam_b", [NUM_LAYERS, NPAD], f32, isOutput=False
        )
        extra_d["bet_b"] = nc.declare_dram_parameter(
            "bet_b", [NUM_LAYERS, NPAD], f32, isOutput=False
        )
    if meta["need_bout"]:
        extra_d["bout"] = nc.declare_dram_parameter("bout", [OUT_DIM, 1], f32, isOutput=False)

    blk_idx = meta["blk_idx"]

    with tile.TileContext(nc) as tc:
        with (
            tc.tile_pool(name="consts", bufs=1) as cpool,
            tc.tile_pool(name="x", bufs=8) as xpool,
            tc.tile_pool(name="vt", bufs=3) as vpool,
            tc.tile_pool(name="r", bufs=4) as rpool,
            tc.tile_pool(name="feat", bufs=6) as fpool,
            tc.tile_pool(name="small", bufs=6) as spool,
            tc.tile_pool(name="outs", bufs=4) as opool,
            tc.tile_pool(name="psA", bufs=2, space="PSUM") as psA,
            tc.tile_pool(name="psB", bufs=2, space="PSUM") as psB,
        ):
            # ---- load constants into SBUF once ----
            ablk_sb = cpool.tile(list(consts["ablk"].shape), bf16)
            wgc_sb = cpool.tile([128, NUM_LAYERS * 128], bf16)
            win4_sb = cpool.tile([128, HID], bf16)
            win34_sb = cpool.tile([128, HID], bf16)
            pos34_sb = cpool.tile([128, NPAD], bf16)
            gridall = cpool.tile([128, (S // 4) * NPAD], bf16)
            wout_sb = cpool.tile([HID, OUT_DIM], bf16)
            ident_sb = cpool.tile([128, 128], bf16)
            nc.sync.dma_start(ablk_sb[:], ablk_d[:])
            nc.sync.dma_start(wgc_sb[:], wgc_d[:])
            nc.sync.dma_start(win4_sb[:], win4_d[:])
            nc.sync.dma_start(win34_sb[:], win34_d[:])
            nc.sync.dma_start(pos34_sb[:], pos34_d[:])
            nc.sync.dma_start(gridall[:], grids_d[:])
            nc.sync.dma_start(wout_sb[:], wout_d[:])
            nc.sync.dma_start(ident_sb[:], ident_d[:])
            xtra = {}
            for k, d in extra_d.items():
                t = cpool.tile(list(d.shape), d.dtype)
                nc.sync.dma_start(t[:], d[:])
                xtra[k] = t

            featT4 = None
            for s in range(S):
                # ================= encode =================
                q = s % 4
                if q == 0:
                    g = s // 4
                    featT4 = fpool.tile([128, NPAD], bf16, tag="featT")
                    nc.vector.tensor_scalar(
                        featT4[:], gridall[:, g * NPAD : (g + 1) * NPAD],
                        0.0, None, ALU.is_equal,
                    )
                tp = (96, 0) if q == 3 else None
                x_ps = psB.tile([128, NPAD], f32, tag="psB")
                for c in range(NCH):
                    sl = slice(c * 128, (c + 1) * 128)
                    nc.tensor.matmul(
                        x_ps[:, sl],
                        featT4[32 * q : 32 * q + NUM_COLORS, sl],
                        win4_sb[32 * q : 32 * q + NUM_COLORS, :],
                        start=True, stop=False, tile_position=tp,
                    )
                    nc.tensor.matmul(
                        x_ps[:, sl],
                        pos34_sb[32 * q : 32 * q + 3, sl],
                        win34_sb[32 * q : 32 * q + 3, :],
                        start=False, stop=True, tile_position=tp,
                    )
                x = xpool.tile([128, NPAD], bf16, tag="x")
                nc.scalar.activation(x[:], x_ps[:], AF.Relu)

                # ================= layers =================
                for l in range(NUM_LAYERS):
                    vt_ps = psA.tile([128, NPAD], f32, tag="psA")
                    for Jc in range(NCH):
                        Is = [i for i in (Jc - 1, Jc, Jc + 1) if (i, Jc) in blk_idx]
                        sl = slice(Jc * 128, (Jc + 1) * 128)
                        for k, I in enumerate(Is):
                            bi = blk_idx[(I, Jc)]
                            nc.tensor.matmul(
                                vt_ps[:, sl],
                                x[:, I * 128 : (I + 1) * 128],
                                ablk_sb[:, bi * 128 : (bi + 1) * 128],
                                start=(k == 0),
                                stop=(k == len(Is) - 1),
                            )
                    vt = vpool.tile([128, NPAD], bf16, tag="vt_a")
                    nc.scalar.activation(vt[:], vt_ps[:], AF.Copy)

                    agg_ps = psB.tile([128, NPAD], f32, tag="psB")
                    for c in range(NCH):
                        sl = slice(c * 128, (c + 1) * 128)
                        nc.tensor.matmul(
                            agg_ps[:, sl],
                            vt[:, sl],
                            wgc_sb[:, l * 128 : (l + 1) * 128],
                            start=True,
                            stop=not meta["need_bg"],
                        )
                        if meta["need_bg"]:
                            nc.tensor.matmul(
                                agg_ps[:, sl],
                                xtra["ones1"][:],
                                xtra["bgc"][l : l + 1, :],
                                start=False,
                                stop=True,
                            )

                    # ---- LN stats: grouped bn_stats (even/odd halves) ----
                    stats = spool.tile([128, NCH, 6], f32, tag="stats")
                    for c in range(NCH):
                        nc.vector.bn_stats(
                            stats[:, c, :], agg_ps[:, c * 128 : (c + 1) * 128]
                        )
                    # var = (S1+S2)/128 + (m1^2+m2^2)/2 - ((m1+m2)/2)^2
                    #     = (S1+S2)/128 + (m1-m2)^2/4
                    dmean = spool.tile([128, NCH], f32, tag="dmean")
                    ssum = spool.tile([128, NCH], f32, tag="ssum")
                    d2 = spool.tile([128, NCH], f32, tag="d2")
                    ve = spool.tile([128, NCH], f32, tag="ve")
                    std = spool.tile([128, NCH], f32, tag="std")
                    rstd = spool.tile([128, NCH], f32, tag="rstd")
                    nc.vector.tensor_tensor(
                        dmean[:], stats[:, :, 1], stats[:, :, 4], ALU.subtract
                    )
                    nc.vector.tensor_tensor(
                        ssum[:], stats[:, :, 2], stats[:, :, 5], ALU.add
                    )
                    nc.vector.tensor_tensor(d2[:], dmean[:], dmean[:], ALU.mult)
                    nc.vector.tensor_scalar(
                        d2[:], d2[:], 0.25, LN_EPS, ALU.mult, ALU.add
                    )
                    nc.vector.tensor_scalar(ssum[:], ssum[:], 1.0 / HID, None, ALU.mult)
                    nc.vector.tensor_tensor(ve[:], ssum[:], d2[:], ALU.add)
                    nc.scalar.activation(std[:], ve[:], AF.Sqrt)
                    nc.vector.reciprocal(rstd[:], std[:])

                    xn = xpool.tile([128, NPAD], bf16, tag="x")
                    if not meta["need_gb"]:
                        # r = relu(rstd*agg_c) == rstd*relu(agg_c);  xn = x + r
                        r = rpool.tile([128, NPAD], bf16, tag="r")
                        for c in range(NCH):
                            sl = slice(c * 128, (c + 1) * 128)
                            nc.scalar.activation(
                                r[:, sl], agg_ps[:, sl], AF.Relu,
                                scale=rstd[:, c : c + 1],
                            )
                        nc.vector.tensor_tensor(xn[:], r[:], x[:], ALU.add)
                    else:
                        # t = agg*rstd*gamma + beta ; xn = x + relu(t)
                        t1 = rpool.tile([128, NPAD], f32, tag="t1")
                        for c in range(NCH):
                            sl = slice(c * 128, (c + 1) * 128)
                            nc.vector.tensor_scalar(
                                t1[:, sl], agg_ps[:, sl], rstd[:, c : c + 1], None, ALU.mult
                            )
                        nc.vector.tensor_tensor(
                            t1[:], t1[:], xtra["gam_b"][l : l + 1, :].partition_broadcast(128), ALU.mult
                        )
                        nc.vector.tensor_tensor(
                            t1[:], t1[:], xtra["bet_b"][l : l + 1, :].partition_broadcast(128), ALU.add
                        )
                        r = rpool.tile([128, NPAD], bf16, tag="r")
                        nc.scalar.activation(r[:], t1[:], AF.Relu)
                        nc.vector.tensor_tensor(xn[:], r[:], x[:], ALU.add)
                    x = xn

                # ================= output =================
                x4t_ps = psA.tile([128, NPAD], bf16, tag="psA")
                for c in range(NCH):
                    sl = slice(c * 128, (c + 1) * 128)
                    nc.tensor.transpose(x4t_ps[:, sl], x[:, sl], ident_sb[:])
                x4t = vpool.tile([128, NPAD], bf16, tag="vt_a")
                nc.scalar.activation(x4t[:], x4t_ps[:], AF.Copy)
                out_ps = psB.tile([OUT_DIM, NPAD], f32, tag="psB")
                for hh in range(2):
                    sl = slice(hh * 512, (hh + 1) * 512)
                    nc.tensor.matmul(
                        out_ps[:, sl], wout_sb[:], x4t[:, sl], start=True, stop=True
                    )
                outs = opool.tile([OUT_DIM, N], f32, tag="outs")
                if meta["need_bout"]:
                    nc.scalar.activation(
                        outs[:], out_ps[:, :N], AF.Identity, bias=xtra["bout"][:]
                    )
                else:
                    nc.scalar.activation(outs[:], out_ps[:, :N], AF.Copy)
                nc.sync.dma_start(out_d[s], outs[:])

    return nc


def _split_excess_waits(nc):
    """walrus's per-instruction sync-command capacity is tiny (an ACTIVATE with
    2 waits + 1 update fails codegen).  Hoist all but one wait of each
    instruction into standalone EventSemaphore waits on the same engine just
    before it -- semantically identical (same-engine program order)."""
    import concourse.mybir as mybir

    n_split = 0
    cnt = [0]

    def mk_wait(engine, w):
        cnt[0] += 1
        es = mybir.InstEventSemaphore(
            name=f"I-wsplit-{cnt[0]}",
            ins=[],
            outs=[],
            sync_info=mybir.SyncInfo(on_wait=[w], on_update=[]),
        )
        es.engine = engine
        return es

    for f in nc.m.functions:
        for blk in f.blocks:
            new = []
            changed = False
            for inst in blk.instructions:
                si = getattr(inst, "sync_info", None)
                waits = list(si.on_wait) if (si is not None and si.on_wait) else []
                if len(waits) > 1 and type(inst).__name__ != "InstAllEngineBarrier":
                    for w in waits[:-1]:
                        new.append(mk_wait(inst.engine, w))
                    inst.sync_info = mybir.SyncInfo(
                        on_wait=[waits[-1]], on_update=list(si.on_update or [])
                    )
                    changed = True
                    n_split += 1
                new.append(inst)
            if changed:
                blk.instructions = new
    return n_split


# --------------------------------------------------------------------------
# Entry point
# --------------------------------------------------------------------------
def _run(inputs, trace=False):
    from concourse.bass_utils import run_bass_kernel_spmd

    g = np.asarray(inputs["grids"]).reshape(B, N).astype(np.float32)
    # packed layout: [128 rows = 4 samples x 32, (S//4)*NPAD]; row 32q+i of
    # col-block grp holds grid[4*grp+q] - i ; pads/unused rows are -1.
    gp = np.full((N_CORES, 128, (S // 4) * NPAD), -1.0, np.float32)
    for core in range(N_CORES):
        for grp in range(S // 4):
            for q in range(4):
                smp = core * S + 4 * grp + q
                for i in range(NUM_COLORS):
                    gp[core, 32 * q + i, grp * NPAD : grp * NPAD + N] = g[smp] - i
    grids = gp.astype(BF16)
    consts, meta = prep_consts(
        inputs["edge_index"],
        inputs["Win"],
        inputs["bin_"],
        inputs["Wg"],
        inputs["bg"],
        inputs["gamma"],
        inputs["beta"],
        inputs["Wout"],
        inputs["bout"],
    )
    nc = build_nc(consts, meta)
    _split_excess_waits(nc)

    common = {k: np.ascontiguousarray(v) for k, v in consts.items()}
    in_maps = []
    for c in range(N_CORES):
        m = dict(common)
        m["grids"] = np.ascontiguousarray(grids[c])
        in_maps.append(m)

    res = run_bass_kernel_spmd(nc, in_maps, list(range(N_CORES)), trace=trace)
    out = np.concatenate([r["out"] for r in res.results], axis=0)  # [B, 64, 900]
    return out.transpose(0, 2, 1).astype(np.float32), res


def kernel(**inputs) -> np.ndarray:
    out, _ = _run(inputs, trace=False)
    return out
